# revision 1
# baseline (speedup 1.0000x reference)
"""NetVLAD Trainium2 Bass kernel, SPMD over 8 NeuronCores.

Contract: kernel(x, Wc, C) takes the FULL inputs
  x  [16, 56, 56, 512] f32, Wc [512, 32] f32, C [512, 32] f32
and returns the FULL output [16, 16384] f32 (matches reference()).

Sharding: data-parallel over batch — 2 samples per core; Wc/C replicated.

Per-core algorithm (3136 pixels/sample, D=512, K=32; 49 tiles of 128
pixels x 512 dims):
  - load x tile [128, 512] (f32r) — contiguous 256KB DMAs
  - PE transpose (4x 128x128 via identity) -> xT in PSUM -> copy to SBUF
  - mm1: s[n,k] = xT.T @ Wc (4 accumulating matmuls, f32r)
  - softmax over k without max-subtraction (|s| <= ~10, exp-safe):
    ACT Exp with fused row-sum accumulator, DVE reciprocal + scale
  - mm2: axT[k,d] += a.T @ x (a stationary 32 cols, x streams N=512, f32r)
    and a_sum[k] += a.T @ ones into a second PSUM bank
  - per-sample epilogue: vT = axT + C^T * a_sum, PE-transpose back to
    [d,k], fused intra+global L2 normalization (the global L2 norm of the
    intra-normalized matrix is exactly sqrt(512), folded analytically),
    DMA out
float32r is used on the matmul paths (1 cycle/row at N>=256 vs 4 for
fp32); measured end-to-end relative error vs the fp32 reference ~1e-4.
"""
import sys

if '/opt/trn_rl_repo' not in sys.path:
    sys.path.insert(0, '/opt/trn_rl_repo')

from contextlib import ExitStack

import numpy as np

F32 = None  # filled on first build (lazy imports keep module import cheap)

N_PIX = 3136
N_SAMP = 2
N_ROWS = N_PIX * N_SAMP
P = 128
NT = N_ROWS // P      # 49
D = 512
K = 32
DC = D // P           # 4
BOUND_T = N_PIX // P  # 24
BOUND_R = N_PIX - BOUND_T * P  # 64
N_CORES = 8

_cache = {}


def _build(use_f32r=True, copy_split=3):
    import concourse.bacc as bacc
    import concourse.mybir as mybir
    import concourse.tile as tile
    from concourse.bass import ts

    F32 = mybir.dt.float32
    F32R = mybir.dt.float32r
    DT = F32R if use_f32r else F32

    nc = bacc.Bacc("TRN2", target_bir_lowering=False, debug=False)

    x = nc.declare_dram_parameter("x", [N_ROWS, D], DT, isOutput=False)
    wc = nc.declare_dram_parameter("wc", [D, K], DT, isOutput=False)
    ct = nc.declare_dram_parameter("ct", [K, D], F32, isOutput=False)
    ident = nc.declare_dram_parameter("ident", [P, P], DT, isOutput=False)
    id32 = nc.declare_dram_parameter("id32", [K, K], F32, isOutput=False)
    ones2 = nc.declare_dram_parameter("ones2", [P, 2], DT, isOutput=False)
    out = nc.declare_dram_parameter("out", [N_SAMP, DC, P, K], F32,
                                    isOutput=True)
    x, wc, ct, ident, out, id32, ones2 = (x.ap(), wc.ap(), ct.ap(),
                                          ident.ap(), out.ap(), id32.ap(),
                                          ones2.ap())

    with tile.TileContext(nc) as tc, ExitStack() as ctx:
        consts = ctx.enter_context(tc.tile_pool(name="consts", bufs=1))
        xpool = ctx.enter_context(tc.tile_pool(name="xpool", bufs=4))
        xtpool = ctx.enter_context(tc.tile_pool(name="xtpool", bufs=3))
        small = ctx.enter_context(tc.tile_pool(name="small", bufs=4))
        epil = ctx.enter_context(tc.tile_pool(name="epil", bufs=2))
        ps_big = ctx.enter_context(tc.tile_pool(name="ps_big", bufs=2,
                                                space="PSUM"))
        ps_sm = ctx.enter_context(tc.tile_pool(name="ps_sm", bufs=2,
                                               space="PSUM"))
        ps_acc = ctx.enter_context(tc.tile_pool(name="ps_acc", bufs=2,
                                                space="PSUM"))

        wc_sb = consts.tile([P, DC, K], DT)
        nc.sync.dma_start(out=wc_sb, in_=wc.rearrange("(c p) k -> p c k", p=P))
        ct_sb = consts.tile([K, D], F32)
        nc.sync.dma_start(out=ct_sb, in_=ct)
        id_sb = consts.tile([P, P], DT)
        nc.sync.dma_start(out=id_sb, in_=ident)
        id32_sb = consts.tile([K, K], F32)
        nc.sync.dma_start(out=id32_sb, in_=id32)
        ones_sb = consts.tile([P, 2], DT)
        nc.sync.dma_start(out=ones_sb, in_=ones2)

        acc = [ps_acc.tile([K, D], F32, name=f"acc{s}", tag="acc")
               for s in range(N_SAMP)]
        asum_ps = [ps_acc.tile([K, 2], F32, name=f"asumps{s}", tag="asum_ps")
                   for s in range(N_SAMP)]
        started = [False, False]

        def epilogue(s):
            asum_sb = epil.tile([K, 1], F32, name=f"asum{s}", tag="asum")
            nc.vector.tensor_copy(asum_sb, asum_ps[s][:, 0:1])
            vt_sb = epil.tile([K, D], F32, name=f"vt{s}", tag="vt")
            nc.vector.tensor_scalar_mul(vt_sb, ct_sb, asum_sb)
            nc.vector.tensor_add(vt_sb, vt_sb, acc[s][:, :])
            v_ps = ps_sm.tile([P, DC, K], F32, name=f"vps{s}", tag="sps")
            for j in range(DC):
                nc.tensor.transpose(v_ps[:, j, :], vt_sb[:, ts(j, P)], id32_sb)
            v_sb = epil.tile([P, DC, K], F32, name=f"v{s}", tag="v")
            nc.vector.tensor_copy(v_sb, v_ps)
            vsq = epil.tile([P, DC, K], F32, name=f"vsq{s}", tag="vsq")
            nc.vector.tensor_mul(vsq, v_sb, v_sb)
            ssq = epil.tile([P, DC], F32, name=f"ssq{s}", tag="ssq")
            nc.vector.reduce_sum(ssq, vsq, axis=mybir.AxisListType.X)
            snorm = epil.tile([P, DC], F32, name=f"sn{s}", tag="sn")
            nc.scalar.activation(snorm, ssq,
                                 mybir.ActivationFunctionType.Sqrt,
                                 scale=float(D))
            rmult = epil.tile([P, DC], F32, name=f"rm{s}", tag="rm")
            nc.vector.reciprocal(rmult, snorm)
            for j in range(DC):
                nc.vector.tensor_scalar_mul(v_sb[:, j, :], v_sb[:, j, :],
                                            rmult[:, j:j + 1])
            nc.sync.dma_start(out=out[s].rearrange("c p k -> p c k"),
                              in_=v_sb)

        for t in range(NT):
            x_t = xpool.tile([P, D], DT, name="x_t")
            nc.sync.dma_start(out=x_t, in_=x[ts(t, P), :])

            xT_ps = ps_big.tile([P, DC, P], DT, name="xT_ps")
            for j in range(DC):
                nc.tensor.transpose(xT_ps[:, j, :], x_t[:, ts(j, P)], id_sb)
            xT_sb = xtpool.tile([P, DC, P], DT, name="xT_sb")
            if copy_split > 0:
                nc.vector.tensor_copy(xT_sb[:, 0:copy_split, :],
                                      xT_ps[:, 0:copy_split, :])
            if copy_split < DC:
                nc.scalar.copy(xT_sb[:, copy_split:DC, :],
                               xT_ps[:, copy_split:DC, :])

            s_ps = ps_sm.tile([P, K], F32, name="s_ps", tag="sps")
            for j in range(DC):
                nc.tensor.matmul(s_ps, xT_sb[:, j, :], wc_sb[:, j, :],
                                 start=(j == 0), stop=(j == DC - 1))

            exp_sb = small.tile([P, K], F32, name="exp_sb")
            sumx = small.tile([P, 1], F32, name="sumx")
            nc.scalar.activation(exp_sb, s_ps,
                                 mybir.ActivationFunctionType.Exp,
                                 accum_out=sumx)
            rcp = small.tile([P, 1], F32, name="rcp")
            nc.vector.reciprocal(rcp, sumx)
            a_sb = small.tile([P, K], DT, name="a_sb")
            nc.vector.tensor_scalar_mul(a_sb, exp_sb, rcp)

            if t < BOUND_T:
                parts = [(0, 0, P)]
            elif t == BOUND_T:
                parts = [(0, 0, BOUND_R), (1, BOUND_R, P)]
            else:
                parts = [(1, 0, P)]
            for s, r0, r1 in parts:
                first = not started[s]
                started[s] = True
                last_tile = (t == BOUND_T and s == 0) or \
                            (t == NT - 1 and s == 1)
                nc.tensor.matmul(acc[s][:, :], a_sb[r0:r1, :], x_t[r0:r1, :],
                                 start=first, stop=last_tile,
                                 skip_group_check=True)
                nc.tensor.matmul(asum_ps[s][:, :], a_sb[r0:r1, :],
                                 ones_sb[r0:r1, :],
                                 start=first, stop=last_tile,
                                 skip_group_check=True)
                if last_tile:
                    epilogue(s)

    nc.finalize()
    return nc


def _get_nc():
    if "nc" not in _cache:
        _cache["nc"] = _build()
    return _cache["nc"]


def kernel(x, Wc, C):
    from concourse.bass_utils import run_bass_kernel_spmd

    nc = _get_nc()

    x = np.asarray(x, dtype=np.float32)
    Wc = np.ascontiguousarray(np.asarray(Wc, dtype=np.float32))
    ct = np.ascontiguousarray(np.asarray(C, dtype=np.float32).T)
    ident = np.eye(P, dtype=np.float32)
    id32 = np.eye(K, dtype=np.float32)
    ones2 = np.ones((P, 2), dtype=np.float32)

    B = x.shape[0]
    per = B // N_CORES
    maps = []
    for i in range(N_CORES):
        xs = np.ascontiguousarray(
            x[i * per:(i + 1) * per].reshape(N_ROWS, D))
        maps.append({"x": xs, "wc": Wc, "ct": ct, "ident": ident,
                     "id32": id32, "ones2": ones2})

    res = run_bass_kernel_spmd(nc, maps, list(range(N_CORES)))
    outs = [r["out"].reshape(N_SAMP, D * K) for r in res.results]
    return np.concatenate(outs, axis=0)



# revision 6
# speedup vs baseline: 2.0353x; 2.0353x over previous
"""NetVLAD Trainium2 Bass kernel, SPMD over 8 NeuronCores.

Contract: kernel(x, Wc, C) takes the FULL inputs
  x  [16, 56, 56, 512] f32, Wc [512, 32] f32, C [512, 32] f32
and returns the FULL output [16, 16384] f32 (matches reference()).

Sharding: data-parallel over batch — 2 samples per core; Wc/C replicated.

v2 design (vs the transpose-on-PE baseline):
  - x is uploaded TWICE, both in bf16: pixel-major xb [6272, 512] (moving
    operand of mm2) and host-pre-transposed xt tiles [49, 128, 4*128]
    with xt[t, p, (j q)] = x[128t+q, 128j+p] (stationary operand of mm1).
    This removes all PE transposes and PSUM->SBUF copies from the main
    loop; bf16 halves DMA vs f32 (12.8 MB/core total, ~36 us at 358 GB/s).
  - mm1 stationary is a 128x128 bf16 tile -> Fast Weight Load applies;
    mm2 stationary is a_sb [128, 32] bf16 (27 ns LDW), moving xb streams
    N=512 at 1 cycle/row. All-matmul PE stream keeps the HAM clock warm.
  - tile DMAs are batched (1 + 6x8 tiles per stream) so each transfer is
    0.1-1 MB and completion latency amortizes.
  - mm2 emission lags mm1 by LAG tiles so the PE queue never waits on the
    softmax chain (ACT exp -> DVE reciprocal -> DVE scale) of the same tile.
  - softmax skips max-subtraction (|s| <= ~10 is exp-safe in f32).
  - epilogue per sample: vT = acc + C^T * a_sum, PE-transpose back to
    [d, k], fused intra+global L2 norm (global norm of the intra-normalized
    matrix is exactly sqrt(512), folded analytically into the Sqrt scale).
Measured end-to-end relative error vs the f32 reference ~2e-3 (bf16 data
path), well under the 2e-2 gate.
"""
import sys

if '/opt/trn_rl_repo' not in sys.path:
    sys.path.insert(0, '/opt/trn_rl_repo')

from contextlib import ExitStack

import numpy as np

N_PIX = 3136
N_SAMP = 2
N_ROWS = N_PIX * N_SAMP
P = 128
NT = N_ROWS // P      # 49
D = 512
K = 32
DC = D // P           # 4
BOUND_T = N_PIX // P  # 24
BOUND_R = N_PIX - BOUND_T * P  # 64
N_CORES = 8
BATCHES = [(0, 1)] + [(1 + 8 * i, 8) for i in range(6)]  # (t0, sz), sum=49
MAXB = 8
LAG = 2

_cache = {}


def _build():
    import concourse.bacc as bacc
    import concourse.mybir as mybir
    import concourse.tile as tile
    from concourse.bass import ts

    F32 = mybir.dt.float32
    BF16 = mybir.dt.bfloat16

    nc = bacc.Bacc("TRN2", target_bir_lowering=False, debug=False)

    xb = nc.declare_dram_parameter("xb", [N_ROWS, D], BF16, isOutput=False)
    xt = nc.declare_dram_parameter("xt", [NT, P, DC, P], BF16,
                                   isOutput=False)
    wc = nc.declare_dram_parameter("wc", [D, K], BF16, isOutput=False)
    ct = nc.declare_dram_parameter("ct", [K, D], F32, isOutput=False)
    id32 = nc.declare_dram_parameter("id32", [K, K], F32, isOutput=False)
    ones2 = nc.declare_dram_parameter("ones2", [P, 2], BF16, isOutput=False)
    out = nc.declare_dram_parameter("out", [N_SAMP, DC, P, K], F32,
                                    isOutput=True)
    xb, xt, wc, ct, id32, ones2, out = (xb.ap(), xt.ap(), wc.ap(), ct.ap(),
                                        id32.ap(), ones2.ap(), out.ap())
    xb_r = xb.rearrange("(t p) d -> p t d", p=P)      # [P, NT, D]
    xt_r = xt.rearrange("t p j q -> p t j q")         # [P, NT, DC, P]

    with tile.TileContext(nc) as tc, ExitStack() as ctx:
        consts = ctx.enter_context(tc.tile_pool(name="consts", bufs=1))
        xbpool = ctx.enter_context(tc.tile_pool(name="xbpool", bufs=3))
        xtpool = ctx.enter_context(tc.tile_pool(name="xtpool", bufs=3))
        small = ctx.enter_context(tc.tile_pool(name="small", bufs=6))
        epil = ctx.enter_context(tc.tile_pool(name="epil", bufs=2))
        ps_s = ctx.enter_context(tc.tile_pool(name="ps_s", bufs=3,
                                              space="PSUM"))
        ps_acc = ctx.enter_context(tc.tile_pool(name="ps_acc", bufs=2,
                                                space="PSUM"))
        ps_asum = ctx.enter_context(tc.tile_pool(name="ps_asum", bufs=2,
                                                 space="PSUM"))

        wc_sb = consts.tile([P, DC, K], BF16)
        nc.sync.dma_start(out=wc_sb, in_=wc.rearrange("(c p) k -> p c k", p=P))
        ct_sb = consts.tile([K, D], F32)
        nc.sync.dma_start(out=ct_sb, in_=ct)
        id32_sb = consts.tile([K, K], F32)
        nc.sync.dma_start(out=id32_sb, in_=id32)
        ones_sb = consts.tile([P, 2], BF16)
        nc.sync.dma_start(out=ones_sb, in_=ones2)

        acc = [ps_acc.tile([K, D], F32, name=f"acc{s}", tag="acc")
               for s in range(N_SAMP)]
        asum_ps = [ps_asum.tile([K, 2], F32, name=f"asumps{s}", tag="asum_ps")
                   for s in range(N_SAMP)]
        started = [False, False]

        def epilogue(s):
            asum_sb = epil.tile([K, 1], F32, name=f"asum{s}", tag="asum")
            nc.vector.tensor_copy(asum_sb, asum_ps[s][:, 0:1])
            vt_sb = epil.tile([K, D], F32, name=f"vt{s}", tag="vt")
            nc.vector.tensor_scalar_mul(vt_sb, ct_sb, asum_sb)
            nc.vector.tensor_add(vt_sb, vt_sb, acc[s][:, :])
            v_ps = ps_s.tile([P, DC, K], F32, name=f"vps{s}", tag="sps")
            for j in range(DC):
                nc.tensor.transpose(v_ps[:, j, :], vt_sb[:, ts(j, P)], id32_sb)
            v_sb = epil.tile([P, DC, K], F32, name=f"v{s}", tag="v")
            nc.vector.tensor_copy(v_sb, v_ps)
            vsq = epil.tile([P, DC, K], F32, name=f"vsq{s}", tag="vsq")
            nc.vector.tensor_mul(vsq, v_sb, v_sb)
            ssq = epil.tile([P, DC], F32, name=f"ssq{s}", tag="ssq")
            nc.vector.reduce_sum(ssq, vsq, axis=mybir.AxisListType.X)
            snorm = epil.tile([P, DC], F32, name=f"sn{s}", tag="sn")
            nc.scalar.activation(snorm, ssq,
                                 mybir.ActivationFunctionType.Sqrt,
                                 scale=float(D))
            rmult = epil.tile([P, DC], F32, name=f"rm{s}", tag="rm")
            nc.vector.reciprocal(rmult, snorm)
            for j in range(DC):
                nc.vector.tensor_scalar_mul(v_sb[:, j, :], v_sb[:, j, :],
                                            rmult[:, j:j + 1])
            nc.sync.dma_start(out=out[s].rearrange("c p k -> p c k"),
                              in_=v_sb)

        def emit_mm2(t, a_sb, xb_t, tt):
            if t < BOUND_T:
                parts = [(0, 0, P)]
            elif t == BOUND_T:
                parts = [(0, 0, BOUND_R), (1, BOUND_R, P)]
            else:
                parts = [(1, 0, P)]
            for s, r0, r1 in parts:
                first = not started[s]
                started[s] = True
                last_tile = (t == BOUND_T and s == 0) or \
                            (t == NT - 1 and s == 1)
                nc.tensor.matmul(acc[s][:, :], a_sb[r0:r1, :],
                                 xb_t[r0:r1, tt, :],
                                 start=first, stop=last_tile,
                                 skip_group_check=True)
                nc.tensor.matmul(asum_ps[s][:, :], a_sb[r0:r1, :],
                                 ones_sb[r0:r1, :],
                                 start=first, stop=last_tile,
                                 skip_group_check=True)
                if last_tile:
                    epilogue(s)

        pending = []
        for t0, sz in BATCHES:
            xb_t = xbpool.tile([P, MAXB, D], BF16, name="xb_t")
            nc.sync.dma_start(out=xb_t[:, 0:sz, :],
                              in_=xb_r[:, t0:t0 + sz, :])
            xt_t = xtpool.tile([P, MAXB, DC, P], BF16, name="xt_t")
            nc.sync.dma_start(out=xt_t[:, 0:sz, :, :],
                              in_=xt_r[:, t0:t0 + sz, :, :])
            for tt in range(sz):
                t = t0 + tt
                s_ps = ps_s.tile([P, K], F32, name="s_ps", tag="sps")
                for j in range(DC):
                    nc.tensor.matmul(s_ps, xt_t[:, tt, j, :], wc_sb[:, j, :],
                                     start=(j == 0), stop=(j == DC - 1))
                exp_sb = small.tile([P, K], F32, name="exp_sb")
                sumx = small.tile([P, 1], F32, name="sumx")
                nc.scalar.activation(exp_sb, s_ps,
                                     mybir.ActivationFunctionType.Exp,
                                     accum_out=sumx)
                rcp = small.tile([P, 1], F32, name="rcp")
                nc.vector.reciprocal(rcp, sumx)
                a_sb = small.tile([P, K], BF16, name="a_sb")
                nc.vector.tensor_scalar_mul(a_sb, exp_sb, rcp)
                pending.append((t, a_sb, xb_t, tt))
                if len(pending) > LAG:
                    emit_mm2(*pending.pop(0))
        for args in pending:
            emit_mm2(*args)

    nc.finalize()
    return nc


def _get_nc():
    if "nc" not in _cache:
        _cache["nc"] = _build()
    return _cache["nc"]


def make_maps(x, Wc, C):
    """Host-side prep: shard over batch, build bf16 xb / pre-transposed xt."""
    import ml_dtypes

    bf16 = ml_dtypes.bfloat16
    x = np.asarray(x, dtype=np.float32)
    wc_h = np.asarray(Wc, dtype=np.float32).astype(bf16)
    ct_h = np.ascontiguousarray(np.asarray(C, dtype=np.float32).T)
    id32 = np.eye(K, dtype=np.float32)
    ones2 = np.ones((P, 2), dtype=bf16)

    B = x.shape[0]
    per = B // N_CORES
    maps = []
    for i in range(N_CORES):
        xs = x[i * per:(i + 1) * per].reshape(N_ROWS, D).astype(bf16)
        # xt[t, p, j, q] = xs[128t+q, 128j+p]
        xtt = np.ascontiguousarray(
            xs.reshape(NT, P, DC, P).transpose(0, 3, 2, 1))
        maps.append({"xb": np.ascontiguousarray(xs), "xt": xtt,
                     "wc": wc_h, "ct": ct_h, "id32": id32, "ones2": ones2})
    return maps


def kernel(x, Wc, C):
    from concourse.bass_utils import run_bass_kernel_spmd

    nc = _get_nc()
    maps = make_maps(x, Wc, C)
    res = run_bass_kernel_spmd(nc, maps, list(range(N_CORES)))
    outs = [r["out"].reshape(N_SAMP, D * K) for r in res.results]
    return np.concatenate(outs, axis=0)
